# revision 5
# baseline (speedup 1.0000x reference)
"""GCN message-passing kernel for 8 trn2 NeuronCores.

Math (per reference): h = relu(a @ (x @ W1) + b1); out = h @ W2 + b2
Shapes: x [8,4096,240], a [4096,4096], W1 [240,32], W2 [32,240].

Sharding: 2x4 grid. Core c -> batch group g=c//4 (4 batches), output-row
group j=c%4 (1024 rows). Each core:
  phase 1: hT_all[128,4096] = stacked (x[b] @ W1).T for 4 batches, computed
           with W1 zero-padded into per-batch column blocks so one PSUM
           accumulation group folds all 4 batches onto 128 partitions.
  transpose: PE-transpose hT_all into h natural layout (128-col blocks).
  phase 2: pa[128,512] = h_block.T @ aT_tile accumulated over 32 k-tiles;
           128 partitions = (batch, hidden) pairs.
  phase 3: relu+b1 on ACT, head matmul vs W2, +b2, DMA out.
"""

import sys

if "/opt/trn_rl_repo" not in sys.path:
    sys.path.insert(0, "/opt/trn_rl_repo")

import numpy as np

B, N, F, H, L = 8, 4096, 240, 32, 240
NB = 4        # batches per core
NRC = 1024    # output rows per core
TRACE = False

_cache = {}
last_exec_time_ns = None
last_profile_json = None


def _install_ntff_hook():
    import types

    import antenv

    if "antenv.axon_hooks" in sys.modules:
        return
    mod = types.ModuleType("antenv.axon_hooks")
    _state = {"hook": None}
    mod.set_axon_ntff_profile_hook = lambda h: _state.__setitem__("hook", h)
    mod.get_axon_ntff_profile_hook = lambda: _state["hook"]
    sys.modules["antenv.axon_hooks"] = mod
    antenv.axon_hooks = mod
    from trn_agent_boot.trn_boot import _ntff_profile_via_ctypes

    mod.set_axon_ntff_profile_hook(
        _ntff_profile_via_ctypes("/opt/axon/libaxon_pjrt.so")
    )


def _build():
    import concourse.bass as bass
    import concourse.tile as tile
    from concourse import bacc, mybir

    f32 = mybir.dt.float32
    ts, ds = bass.ts, bass.ds

    nc = bacc.Bacc("TRN2", target_bir_lowering=False, debug=False, num_devices=8)
    xT = nc.dram_tensor("xT", [NB * F, N], f32, kind="ExternalInput").ap()
    aT = nc.dram_tensor("aT", [N, NRC], f32, kind="ExternalInput").ap()
    w1p = nc.dram_tensor("w1p", [F, 512], f32, kind="ExternalInput").ap()
    w2k = nc.dram_tensor("w2k", [128, 960], f32, kind="ExternalInput").ap()
    b1s = nc.dram_tensor("b1s", [128, 1], f32, kind="ExternalInput").ap()
    b2k = nc.dram_tensor("b2k", [128, 960], f32, kind="ExternalInput").ap()
    idn = nc.dram_tensor("idn", [128, 128], f32, kind="ExternalInput").ap()
    out = nc.dram_tensor("out", [NB * NRC, L], f32, kind="ExternalOutput").ap()

    relu = mybir.ActivationFunctionType.Relu

    with tile.TileContext(nc) as tc:
        with tc.tile_pool(name="const", bufs=1) as cp:
            w1a = cp.tile([128, 512], f32)
            nc.sync.dma_start(w1a[:], w1p[0:128, :])
            w1b = cp.tile([112, 512], f32)
            nc.sync.dma_start(w1b[:], w1p[128:240, :])
            w2s = cp.tile([128, 960], f32)
            nc.sync.dma_start(w2s[:], w2k[:])
            b1t = cp.tile([128, 1], f32)
            nc.sync.dma_start(b1t[:], b1s[:])
            b2t = cp.tile([128, 960], f32)
            nc.sync.dma_start(b2t[:], b2k[:])
            idt = cp.tile([128, 128], f32)
            nc.sync.dma_start(idt[:], idn[:])
            hT = cp.tile([128, N], f32)
            hsb = cp.tile([128, N], f32)

            # phase 1: hT[32b+h, n] = sum_f W1[f,h] * x[b,n,f]
            with tc.tile_pool(name="xs", bufs=3) as xs, \
                 tc.tile_pool(name="ps1", bufs=2, space="PSUM") as ps1:
                for ncol in range(8):
                    p1 = ps1.tile([128, 512], f32)
                    for b in range(NB):
                        xa = xs.tile([128, 512], f32)
                        nc.sync.dma_start(
                            xa[:], xT[ds(b * F, 128), ts(ncol, 512)])
                        xb = xs.tile([112, 512], f32)
                        nc.sync.dma_start(
                            xb[:], xT[ds(b * F + 128, 112), ts(ncol, 512)])
                        nc.tensor.matmul(p1[:], w1a[:, ts(b, 128)], xa[:],
                                         start=(b == 0), stop=False)
                        nc.tensor.matmul(p1[:], w1b[:, ts(b, 128)], xb[:],
                                         start=False, stop=(b == NB - 1))
                    nc.vector.tensor_copy(hT[:, ts(ncol, 512)], p1[:])

            # transpose hT -> hsb: hsb[p, 128m + 32b + h] = h[b][128m+p, h]
            with tc.tile_pool(name="pst", bufs=2, space="PSUM") as pst:
                for m in range(32):
                    pt = pst.tile([128, 128], f32)
                    nc.tensor.transpose(pt[:], hT[:, ts(m, 128)], idt[:])
                    nc.vector.tensor_copy(hsb[:, ts(m, 128)], pt[:])

            # phase 2 + 3
            with tc.tile_pool(name="ap_", bufs=4) as ap_, \
                 tc.tile_pool(name="rs", bufs=2) as rs, \
                 tc.tile_pool(name="os", bufs=3) as osb, \
                 tc.tile_pool(name="ps2", bufs=2, space="PSUM") as ps2, \
                 tc.tile_pool(name="ps3", bufs=2, space="PSUM") as ps3:
                for mc in range(2):
                    pa = ps2.tile([128, 512], f32)
                    for kt in range(32):
                        at = ap_.tile([128, 512], f32)
                        nc.sync.dma_start(at[:], aT[ts(kt, 128), ts(mc, 512)])
                        nc.tensor.matmul(pa[:], hsb[:, ts(kt, 128)], at[:],
                                         start=(kt == 0), stop=(kt == 31))
                    r = rs.tile([128, 512], f32)
                    nc.scalar.activation(r[:], pa[:], relu, bias=b1t[:])
                    # head: out cols as (b, l) pairs via block-diagonal W2.
                    # w2s[32b+h, hf*480 + b*120 + li] = W2[h, hf*120 + li]
                    for s in range(4):
                        o = osb.tile([128, NB * L], f32)
                        ov = o[:].rearrange("p (b l) -> p b l", b=NB)
                        for hf in range(2):
                            p3 = ps3.tile([128, 480], f32)
                            nc.tensor.matmul(
                                p3[:], r[:, ts(s, 128)], w2s[:, ts(hf, 480)],
                                start=True, stop=True)
                            nc.vector.tensor_add(
                                ov[:, :, ds(hf * 120, 120)],
                                p3[:].rearrange("p (b l) -> p b l", b=NB),
                                b2t[:, ts(hf, 480)].rearrange(
                                    "p (b l) -> p b l", b=NB))
                        for b in range(NB):
                            nc.sync.dma_start(
                                out[ds(b * NRC + mc * 512 + s * 128, 128), :],
                                o[:, ds(b * L, L)])

    nc.compile()
    return nc


def kernel(x, a, W1, b1, W2, b2):
    global last_exec_time_ns, last_profile_json
    from concourse.bass_utils import run_bass_kernel_spmd

    if "nc" not in _cache:
        _cache["nc"] = _build()
    nc = _cache["nc"]

    x = np.asarray(x, np.float32)
    a = np.asarray(a, np.float32)
    W1 = np.asarray(W1, np.float32)
    b1 = np.asarray(b1, np.float32)
    W2 = np.asarray(W2, np.float32)
    b2 = np.asarray(b2, np.float32)

    xg = [np.ascontiguousarray(
        x[g * NB:(g + 1) * NB].transpose(0, 2, 1)).reshape(NB * F, N)
        for g in range(2)]
    aj = [np.ascontiguousarray(a[j * NRC:(j + 1) * NRC, :].T)
          for j in range(4)]
    w1p = np.zeros((F, 512), np.float32)
    for b in range(NB):
        w1p[:, 128 * b + 32 * b:128 * b + 32 * b + 32] = W1
    # w2k[32b+h, hf*480 + b*120 + li] = W2[h, hf*120 + li]; zeros elsewhere
    w2k = np.zeros((128, 960), np.float32)
    b2k = np.empty((128, 960), np.float32)
    for hf in range(2):
        for b in range(NB):
            w2k[32 * b:32 * b + 32, 480 * hf + 120 * b:480 * hf + 120 * b + 120] = \
                W2[:, 120 * hf:120 * hf + 120]
            b2k[:, 480 * hf + 120 * b:480 * hf + 120 * b + 120] = \
                b2[None, 120 * hf:120 * hf + 120]
    b1s = np.ascontiguousarray(np.tile(b1, 4).reshape(128, 1))
    idn = np.eye(128, dtype=np.float32)

    ins = []
    for c in range(8):
        g, j = c // 4, c % 4
        ins.append({"xT": xg[g], "aT": aj[j], "w1p": w1p, "w2k": w2k,
                    "b1s": b1s, "b2k": b2k, "idn": idn})

    trace = TRACE
    if trace:
        try:
            _install_ntff_hook()
        except Exception:
            trace = False
    r = run_bass_kernel_spmd(nc, ins, list(range(8)), trace=trace)
    last_exec_time_ns = r.exec_time_ns
    last_profile_json = r.profile_json

    res = np.empty((B, N, L), np.float32)
    for c in range(8):
        g, j = c // 4, c % 4
        res[g * NB:(g + 1) * NB, j * NRC:(j + 1) * NRC, :] = \
            r.results[c]["out"].reshape(NB, NRC, L)
    return res


# revision 6
# speedup vs baseline: 1.4944x; 1.4944x over previous
"""GCN message-passing kernel for 8 trn2 NeuronCores.

Math (per reference): h = relu(a @ (x @ W1) + b1); out = h @ W2 + b2
Shapes: x [8,4096,240], a [4096,4096], W1 [240,32], W2 [32,240].

Sharding: 2x4 grid. Core c -> batch group g=c//4 (4 batches), output-row
group j=c%4 (1024 rows). Inputs x, a, W1, W2 are host-converted to fp16
(PE runs fp16 at 1 pass/col vs fp32's 2; DMA traffic halves). PSUM
accumulation stays fp32; measured end-to-end rel err ~3e-4.

Per core:
  phase 1: hT_all[128,4096] = stacked (x[b] @ W1).T for 4 batches, computed
           with W1 zero-padded into per-batch column blocks so one PSUM
           accumulation group folds all 4 batches onto 128 partitions.
  transpose: PE-transpose hT_all into h natural layout (128-col blocks).
  phase 2: pa[128,512] = h_block.T @ aT_tile accumulated over 32 k-tiles;
           128 partitions = (batch, hidden) pairs.
  phase 3: relu+b1 on ACT (fp16 out), head matmul vs W2, +b2, DMA out.
"""

import sys

if "/opt/trn_rl_repo" not in sys.path:
    sys.path.insert(0, "/opt/trn_rl_repo")

import numpy as np

B, N, F, H, L = 8, 4096, 240, 32, 240
NB = 4        # batches per core
NRC = 1024    # output rows per core
TRACE = False

_cache = {}
last_exec_time_ns = None
last_profile_json = None


def _install_ntff_hook():
    import types

    import antenv

    if "antenv.axon_hooks" in sys.modules:
        return
    mod = types.ModuleType("antenv.axon_hooks")
    _state = {"hook": None}
    mod.set_axon_ntff_profile_hook = lambda h: _state.__setitem__("hook", h)
    mod.get_axon_ntff_profile_hook = lambda: _state["hook"]
    sys.modules["antenv.axon_hooks"] = mod
    antenv.axon_hooks = mod
    from trn_agent_boot.trn_boot import _ntff_profile_via_ctypes

    mod.set_axon_ntff_profile_hook(
        _ntff_profile_via_ctypes("/opt/axon/libaxon_pjrt.so")
    )


def _build():
    import concourse.bass as bass
    import concourse.tile as tile
    from concourse import bacc, mybir

    f32 = mybir.dt.float32
    f16 = mybir.dt.float16
    ts, ds = bass.ts, bass.ds

    nc = bacc.Bacc("TRN2", target_bir_lowering=False, debug=False, num_devices=8)
    xT = nc.dram_tensor("xT", [NB * F, N], f16, kind="ExternalInput").ap()
    aT = nc.dram_tensor("aT", [N, NRC], f16, kind="ExternalInput").ap()
    w1p = nc.dram_tensor("w1p", [F, 512], f16, kind="ExternalInput").ap()
    w2k = nc.dram_tensor("w2k", [128, 960], f16, kind="ExternalInput").ap()
    b1s = nc.dram_tensor("b1s", [128, 1], f32, kind="ExternalInput").ap()
    b2k = nc.dram_tensor("b2k", [128, 960], f32, kind="ExternalInput").ap()
    idn = nc.dram_tensor("idn", [128, 128], f16, kind="ExternalInput").ap()
    out = nc.dram_tensor("out", [NB * NRC, L], f32, kind="ExternalOutput").ap()

    relu = mybir.ActivationFunctionType.Relu

    with tile.TileContext(nc) as tc:
        with tc.tile_pool(name="const", bufs=1) as cp:
            w1a = cp.tile([128, 512], f16)
            nc.sync.dma_start(w1a[:], w1p[0:128, :])
            w1b = cp.tile([112, 512], f16)
            nc.sync.dma_start(w1b[:], w1p[128:240, :])
            w2s = cp.tile([128, 960], f16)
            nc.sync.dma_start(w2s[:], w2k[:])
            b1t = cp.tile([128, 1], f32)
            nc.sync.dma_start(b1t[:], b1s[:])
            b2t = cp.tile([128, 960], f32)
            nc.sync.dma_start(b2t[:], b2k[:])
            idt = cp.tile([128, 128], f16)
            nc.sync.dma_start(idt[:], idn[:])
            hT = cp.tile([128, N], f16)
            hsb = cp.tile([128, N], f16)

            # phase 1: hT[32b+h, n] = sum_f W1[f,h] * x[b,n,f]
            with tc.tile_pool(name="xs", bufs=3) as xs, \
                 tc.tile_pool(name="ps1", bufs=2, space="PSUM") as ps1:
                for ncol in range(8):
                    p1 = ps1.tile([128, 512], f32)
                    for b in range(NB):
                        xa = xs.tile([128, 512], f16)
                        nc.sync.dma_start(
                            xa[:], xT[ds(b * F, 128), ts(ncol, 512)])
                        xb = xs.tile([112, 512], f16)
                        nc.sync.dma_start(
                            xb[:], xT[ds(b * F + 128, 112), ts(ncol, 512)])
                        nc.tensor.matmul(p1[:], w1a[:, ts(b, 128)], xa[:],
                                         start=(b == 0), stop=False)
                        nc.tensor.matmul(p1[:], w1b[:, ts(b, 128)], xb[:],
                                         start=False, stop=(b == NB - 1))
                    nc.vector.tensor_copy(hT[:, ts(ncol, 512)], p1[:])

            # transpose hT -> hsb: hsb[p, 128m + 32b + h] = h[b][128m+p, h]
            with tc.tile_pool(name="pst", bufs=2, space="PSUM") as pst:
                for m in range(32):
                    pt = pst.tile([128, 128], f16)
                    nc.tensor.transpose(pt[:], hT[:, ts(m, 128)], idt[:])
                    nc.vector.tensor_copy(hsb[:, ts(m, 128)], pt[:])

            # phase 2 + 3
            with tc.tile_pool(name="ap_", bufs=4) as ap_, \
                 tc.tile_pool(name="rs", bufs=2) as rs, \
                 tc.tile_pool(name="os", bufs=3) as osb, \
                 tc.tile_pool(name="ps2", bufs=2, space="PSUM") as ps2, \
                 tc.tile_pool(name="ps3", bufs=2, space="PSUM") as ps3:
                for mc in range(2):
                    pa = ps2.tile([128, 512], f32)
                    for kt in range(32):
                        at = ap_.tile([128, 512], f16)
                        nc.sync.dma_start(at[:], aT[ts(kt, 128), ts(mc, 512)])
                        nc.tensor.matmul(pa[:], hsb[:, ts(kt, 128)], at[:],
                                         start=(kt == 0), stop=(kt == 31))
                    r = rs.tile([128, 512], f16)
                    nc.scalar.activation(r[:], pa[:], relu, bias=b1t[:])
                    # head: out cols as (b, l) pairs via block-diagonal W2.
                    # w2s[32b+h, hf*480 + b*120 + li] = W2[h, hf*120 + li]
                    for s in range(4):
                        o = osb.tile([128, NB * L], f32)
                        ov = o[:].rearrange("p (b l) -> p b l", b=NB)
                        for hf in range(2):
                            p3 = ps3.tile([128, 480], f32)
                            nc.tensor.matmul(
                                p3[:], r[:, ts(s, 128)], w2s[:, ts(hf, 480)],
                                start=True, stop=True)
                            nc.vector.tensor_add(
                                ov[:, :, ds(hf * 120, 120)],
                                p3[:].rearrange("p (b l) -> p b l", b=NB),
                                b2t[:, ts(hf, 480)].rearrange(
                                    "p (b l) -> p b l", b=NB))
                        for b in range(NB):
                            nc.sync.dma_start(
                                out[ds(b * NRC + mc * 512 + s * 128, 128), :],
                                o[:, ds(b * L, L)])

    nc.compile()
    return nc


def kernel(x, a, W1, b1, W2, b2):
    global last_exec_time_ns, last_profile_json
    from concourse.bass_utils import run_bass_kernel_spmd

    if "nc" not in _cache:
        _cache["nc"] = _build()
    nc = _cache["nc"]

    x = np.asarray(x, np.float32)
    a = np.asarray(a, np.float32)
    W1 = np.asarray(W1, np.float32)
    b1 = np.asarray(b1, np.float32)
    W2 = np.asarray(W2, np.float32)
    b2 = np.asarray(b2, np.float32)

    xg = [np.ascontiguousarray(
        x[g * NB:(g + 1) * NB].transpose(0, 2, 1)).reshape(
            NB * F, N).astype(np.float16)
        for g in range(2)]
    aj = [np.ascontiguousarray(a[j * NRC:(j + 1) * NRC, :].T).astype(
        np.float16) for j in range(4)]
    w1p = np.zeros((F, 512), np.float16)
    for b in range(NB):
        w1p[:, 128 * b + 32 * b:128 * b + 32 * b + 32] = W1.astype(np.float16)
    # w2k[32b+h, hf*480 + b*120 + li] = W2[h, hf*120 + li]; zeros elsewhere
    w2k = np.zeros((128, 960), np.float16)
    b2k = np.empty((128, 960), np.float32)
    for hf in range(2):
        for b in range(NB):
            w2k[32 * b:32 * b + 32, 480 * hf + 120 * b:480 * hf + 120 * b + 120] = \
                W2[:, 120 * hf:120 * hf + 120].astype(np.float16)
            b2k[:, 480 * hf + 120 * b:480 * hf + 120 * b + 120] = \
                b2[None, 120 * hf:120 * hf + 120]
    b1s = np.ascontiguousarray(np.tile(b1, 4).reshape(128, 1))
    idn = np.eye(128, dtype=np.float16)

    ins = []
    for c in range(8):
        g, j = c // 4, c % 4
        ins.append({"xT": xg[g], "aT": aj[j], "w1p": w1p, "w2k": w2k,
                    "b1s": b1s, "b2k": b2k, "idn": idn})

    trace = TRACE
    if trace:
        try:
            _install_ntff_hook()
        except Exception:
            trace = False
    r = run_bass_kernel_spmd(nc, ins, list(range(8)), trace=trace)
    last_exec_time_ns = r.exec_time_ns
    last_profile_json = r.profile_json

    res = np.empty((B, N, L), np.float32)
    for c in range(8):
        g, j = c // 4, c % 4
        res[g * NB:(g + 1) * NB, j * NRC:(j + 1) * NRC, :] = \
            r.results[c]["out"].reshape(NB, NRC, L)
    return res


# revision 9
# speedup vs baseline: 1.8399x; 1.2312x over previous
"""GCN message-passing kernel for 8 trn2 NeuronCores.

Math (per reference): h = relu(a @ (x @ W1) + b1); out = h @ W2 + b2
Shapes: x [8,4096,240], a [4096,4096], W1 [240,32], W2 [32,240].

Sharding: 2x4 grid. Core c -> batch group g=c//4 (4 batches), output-row
group j=c%4 (1024 rows). Inputs x, a, W1, W2 host-converted to fp16 (1-pass
PE, half DMA); PSUM accumulation fp32; end-to-end rel err ~5e-4.

All DMA transfers are full-row contiguous DRAM extents (xT in ~1MB blocks,
aT rows in 256KB blocks, output written partition-major at 3.8KB/partition)
to avoid the 1KB-packet descriptor overhead that dominated row-strided DMA.

Per core:
  phase 1 (b-outer): hT[32b+h, n] accumulated over 4 batches into 8
          persistent PSUM banks (one per 512-col block of n), with W1
          zero-padded per-batch so 128 partitions = (batch, hidden).
  transpose: PE-transpose hT into h natural layout (128-col blocks).
  phase 2 (kt-outer): two persistent PSUM accumulators, one per 512-row
          output chunk; each aT row-tile [128,1024] feeds both.
  phase 3: relu+b1 on ACT (fp16), block-diagonal W2 head matmul, +b2,
          write to outp[128, 7680] partition-major.
"""

import sys

if "/opt/trn_rl_repo" not in sys.path:
    sys.path.insert(0, "/opt/trn_rl_repo")

import numpy as np

B, N, F, H, L = 8, 4096, 240, 32, 240
NB = 4        # batches per core
NRC = 1024    # output rows per core
TRACE = False

_cache = {}
last_exec_time_ns = None
last_profile_json = None


def _install_ntff_hook():
    import types

    import antenv

    if "antenv.axon_hooks" in sys.modules:
        return
    mod = types.ModuleType("antenv.axon_hooks")
    _state = {"hook": None}
    mod.set_axon_ntff_profile_hook = lambda h: _state.__setitem__("hook", h)
    mod.get_axon_ntff_profile_hook = lambda: _state["hook"]
    sys.modules["antenv.axon_hooks"] = mod
    antenv.axon_hooks = mod
    from trn_agent_boot.trn_boot import _ntff_profile_via_ctypes

    mod.set_axon_ntff_profile_hook(
        _ntff_profile_via_ctypes("/opt/axon/libaxon_pjrt.so")
    )


def _build():
    import concourse.bass as bass
    import concourse.tile as tile
    from concourse import bacc, mybir

    f32 = mybir.dt.float32
    f16 = mybir.dt.float16
    ts, ds = bass.ts, bass.ds

    nc = bacc.Bacc("TRN2", target_bir_lowering=False, debug=False, num_devices=8)
    xT = nc.dram_tensor("xT", [NB * F, N], f16, kind="ExternalInput").ap()
    aT = nc.dram_tensor("aT", [N, NRC], f16, kind="ExternalInput").ap()
    w1p = nc.dram_tensor("w1p", [F, 512], f16, kind="ExternalInput").ap()
    w2k = nc.dram_tensor("w2k", [128, 960], f16, kind="ExternalInput").ap()
    b1s = nc.dram_tensor("b1s", [128, 1], f32, kind="ExternalInput").ap()
    b2k = nc.dram_tensor("b2k", [128, 960], f32, kind="ExternalInput").ap()
    idn = nc.dram_tensor("idn", [128, 128], f16, kind="ExternalInput").ap()
    outp = nc.dram_tensor("outp", [128, 8 * NB * L], f32,
                          kind="ExternalOutput").ap()

    relu = mybir.ActivationFunctionType.Relu

    with tile.TileContext(nc) as tc:
        with tc.tile_pool(name="const", bufs=1) as cp:
            w1a = cp.tile([128, 512], f16)
            nc.sync.dma_start(w1a[:], w1p[0:128, :])
            w1b = cp.tile([112, 512], f16)
            nc.sync.dma_start(w1b[:], w1p[128:240, :])
            w2s = cp.tile([128, 960], f16)
            nc.sync.dma_start(w2s[:], w2k[:])
            b1t = cp.tile([128, 1], f32)
            nc.sync.dma_start(b1t[:], b1s[:])
            b2t = cp.tile([128, 960], f32)
            nc.sync.dma_start(b2t[:], b2k[:])
            idt = cp.tile([128, 128], f16)
            nc.sync.dma_start(idt[:], idn[:])
            hT = cp.tile([128, N], f16)
            hsb = cp.tile([128, N], f16)

            # phase 1: hT[32b+h, n] = sum_f W1[f,h] * x[b,n,f]
            with tc.tile_pool(name="xs", bufs=2) as xs, \
                 tc.tile_pool(name="ps1", bufs=1, space="PSUM") as ps1:
                p1 = [ps1.tile([128, 512], f32, name=f"p1_{i}")
                      for i in range(8)]
                for b in range(NB):
                    xa = xs.tile([128, N], f16)
                    nc.sync.dma_start(xa[:], xT[ds(b * F, 128), :])
                    xb = xs.tile([112, N], f16)
                    nc.sync.dma_start(xb[:], xT[ds(b * F + 128, 112), :])
                    for ncol in range(8):
                        nc.tensor.matmul(
                            p1[ncol][:], w1a[:, ts(b, 128)],
                            xa[:, ts(ncol, 512)],
                            start=(b == 0), stop=False)
                        nc.tensor.matmul(
                            p1[ncol][:], w1b[:, ts(b, 128)],
                            xb[:, ts(ncol, 512)],
                            start=False, stop=(b == NB - 1))
                for ncol in range(8):
                    nc.vector.tensor_copy(hT[:, ts(ncol, 512)], p1[ncol][:])

            # transpose hT -> hsb: hsb[p, 128m + 32b + h] = h[b][128m+p, h]
            with tc.tile_pool(name="pst", bufs=2, space="PSUM") as pst:
                for m in range(32):
                    pt = pst.tile([128, 128], f16)
                    nc.tensor.transpose(pt[:], hT[:, ts(m, 128)], idt[:])
                    nc.vector.tensor_copy(hsb[:, ts(m, 128)], pt[:])

            # phase 2: kt-outer, both 512-col output chunks per aT row-tile
            with tc.tile_pool(name="ap_", bufs=4) as ap_, \
                 tc.tile_pool(name="rs", bufs=2) as rs, \
                 tc.tile_pool(name="os", bufs=3) as osb, \
                 tc.tile_pool(name="ps2", bufs=1, space="PSUM") as ps2, \
                 tc.tile_pool(name="ps3", bufs=2, space="PSUM") as ps3:
                pa = [ps2.tile([128, 512], f32, name=f"pa_{i}")
                      for i in range(2)]
                for kt in range(32):
                    at = ap_.tile([128, NRC], f16)
                    nc.sync.dma_start(at[:], aT[ts(kt, 128), :])
                    for mc in range(2):
                        nc.tensor.matmul(
                            pa[mc][:], hsb[:, ts(kt, 128)],
                            at[:, ts(mc, 512)],
                            start=(kt == 0), stop=(kt == 31))
                for mc in range(2):
                    r = rs.tile([128, 512], f16)
                    nc.scalar.activation(r[:], pa[mc][:], relu, bias=b1t[:])
                    # head: out cols as (b, l) pairs via block-diagonal W2.
                    # w2s[32b+h, hf*480 + b*120 + li] = W2[h, hf*120 + li]
                    for s in range(4):
                        o = osb.tile([128, NB * L], f32)
                        ov = o[:].rearrange("p (b l) -> p b l", b=NB)
                        for hf in range(2):
                            p3 = ps3.tile([128, 480], f32)
                            nc.tensor.matmul(
                                p3[:], r[:, ts(s, 128)], w2s[:, ts(hf, 480)],
                                start=True, stop=True)
                            nc.vector.tensor_add(
                                ov[:, :, ds(hf * 120, 120)],
                                p3[:].rearrange("p (b l) -> p b l", b=NB),
                                b2t[:, ts(hf, 480)].rearrange(
                                    "p (b l) -> p b l", b=NB))
                        nc.sync.dma_start(
                            outp[:, ts(mc * 4 + s, NB * L)], o[:])

    nc.compile()
    return nc


def kernel(x, a, W1, b1, W2, b2):
    global last_exec_time_ns, last_profile_json
    from concourse.bass_utils import run_bass_kernel_spmd

    if "nc" not in _cache:
        _cache["nc"] = _build()
    nc = _cache["nc"]

    x = np.asarray(x, np.float32)
    a = np.asarray(a, np.float32)
    W1 = np.asarray(W1, np.float32)
    b1 = np.asarray(b1, np.float32)
    W2 = np.asarray(W2, np.float32)
    b2 = np.asarray(b2, np.float32)

    xg = [np.ascontiguousarray(
        x[g * NB:(g + 1) * NB].transpose(0, 2, 1)).reshape(
            NB * F, N).astype(np.float16)
        for g in range(2)]
    aj = [np.ascontiguousarray(a[j * NRC:(j + 1) * NRC, :].T).astype(
        np.float16) for j in range(4)]
    w1p = np.zeros((F, 512), np.float16)
    for b in range(NB):
        w1p[:, 128 * b + 32 * b:128 * b + 32 * b + 32] = W1.astype(np.float16)
    # w2k[32b+h, hf*480 + b*120 + li] = W2[h, hf*120 + li]; zeros elsewhere
    w2k = np.zeros((128, 960), np.float16)
    b2k = np.empty((128, 960), np.float32)
    for hf in range(2):
        for b in range(NB):
            w2k[32 * b:32 * b + 32, 480 * hf + 120 * b:480 * hf + 120 * b + 120] = \
                W2[:, 120 * hf:120 * hf + 120].astype(np.float16)
            b2k[:, 480 * hf + 120 * b:480 * hf + 120 * b + 120] = \
                b2[None, 120 * hf:120 * hf + 120]
    b1s = np.ascontiguousarray(np.tile(b1, 4).reshape(128, 1))
    idn = np.eye(128, dtype=np.float16)

    ins = []
    for c in range(8):
        g, j = c // 4, c % 4
        ins.append({"xT": xg[g], "aT": aj[j], "w1p": w1p, "w2k": w2k,
                    "b1s": b1s, "b2k": b2k, "idn": idn})

    trace = TRACE
    if trace:
        try:
            _install_ntff_hook()
        except Exception:
            trace = False
    r = run_bass_kernel_spmd(nc, ins, list(range(8)), trace=trace)
    last_exec_time_ns = r.exec_time_ns
    last_profile_json = r.profile_json

    res = np.empty((B, N, L), np.float32)
    for c in range(8):
        g, j = c // 4, c % 4
        # outp[p, ((mc,s), b, l)] -> n = mc*512 + s*128 + p
        arr = r.results[c]["outp"].reshape(128, 8, NB, L)
        res[g * NB:(g + 1) * NB, j * NRC:(j + 1) * NRC, :] = \
            arr.transpose(2, 1, 0, 3).reshape(NB, NRC, L)
    return res
